# revision 5
# baseline (speedup 1.0000x reference)
"""ChannelAttention TRN2 kernel (8 NeuronCores, data-parallel over batch).

Math: for each batch b with X = x[b] [L, C]:
    S_g = Wqa_g^T G_a Wka_g   with G_a = [X|1]^T [X|1] (Gram, contracts over L,
                              absorbs q/k biases; q-scale folded into Wqa)
    P_g = softmax(S_g)        tiny [64, 64] per group
    O   = X Mv + 1 nv^T       with MvT_g = P_g Wv_g^T  (so O never materializes)
    Y   = X W_eff + 1 b_eff   with W_eff = Mv Wp, b_eff = Wp^T nv + bp
so the [L, 3C] qkv intermediate is never computed; per-core work is two
L-sized GEMMs (G = X^T X and Y = X W_eff) plus [512]-scale matrices.

float32r is used for every N=512 matmul (full-rate fp32 streaming; measured
identical numerics to plain fp32 matmul on TRN2). fp32r operands must be
produced as fp32r, hence the dtype choreography on tiles below.
"""

import numpy as np
from contextlib import ExitStack

import concourse.bacc as bacc
import concourse.bass as bass
import concourse.mybir as mybir
import concourse.tile as tile
from concourse.bass_utils import run_bass_kernel_spmd

B, L, C = 8, 4096, 512
GROUPS, HD = 8, 64
P = 128
NT = L // P          # 32 l-tiles
NJ = C // P          # 4 c-chunks
NCORES = 8
FP32 = mybir.dt.float32
FP32R = mybir.dt.float32r


def _build_nc():
    nc = bacc.Bacc("TRN2", target_bir_lowering=False, debug=False, num_devices=NCORES)

    x_d = nc.dram_tensor("x", [L, C], FP32R, kind="ExternalInput").ap()
    xt_d = nc.dram_tensor("xt", [C, L], FP32R, kind="ExternalInput").ap()
    wqa_d = nc.dram_tensor("wqa", [C + 1, C], FP32, kind="ExternalInput").ap()
    wka_d = nc.dram_tensor("wka", [C + 1, C], FP32R, kind="ExternalInput").ap()
    wvt_d = nc.dram_tensor("wvt", [4, HD, 2 * C], FP32R, kind="ExternalInput").ap()
    bv_d = nc.dram_tensor("bv", [4, HD, 2], FP32, kind="ExternalInput").ap()
    wp_d = nc.dram_tensor("wp", [C, C], FP32R, kind="ExternalInput").ap()
    bp_d = nc.dram_tensor("bp", [1, C], FP32R, kind="ExternalInput").ap()
    id_d = nc.dram_tensor("id128", [P, P], FP32, kind="ExternalInput").ap()
    onesc_d = nc.dram_tensor("onesc", [P, 1], FP32R, kind="ExternalInput").ap()
    # cst row: [0:128]=1.0 (Y-bias lhsT), [128]=L, [129]=1.0
    cst_d = nc.dram_tensor("cst", [1, 130], FP32R, kind="ExternalInput").ap()
    y_d = nc.dram_tensor("y", [L, C], FP32, kind="ExternalOutput").ap()

    with tile.TileContext(nc) as tc, ExitStack() as ctx:
        wpool = ctx.enter_context(tc.tile_pool(name="weights", bufs=1))

        xt_sb = [wpool.tile([P, L], FP32R, name=f"xt_sb{j}", tag=f"xt{j}") for j in range(NJ)]
        wqa_sb = [wpool.tile([P, C], FP32, name=f"wqa_sb{j}", tag=f"wqa{j}") for j in range(NJ)]
        wqa_row = wpool.tile([1, C], FP32, name="wqa_row", tag="wqa_row")
        wka_sb = [wpool.tile([P, C], FP32R, name=f"wka_sb{j}", tag=f"wka{j}") for j in range(NJ)]
        wka_row = wpool.tile([1, C], FP32R, name="wka_row", tag="wka_row")
        wvt_sb = [wpool.tile([HD, 2 * C], FP32R, name=f"wvt_sb{p}", tag=f"wvt{p}") for p in range(4)]
        bv_sb = [wpool.tile([HD, 2], FP32, name=f"bv_sb{p}", tag=f"bv{p}") for p in range(4)]
        wp_sb = [wpool.tile([P, C], FP32R, name=f"wp_sb{j}", tag=f"wp{j}") for j in range(NJ)]
        bp_sb = wpool.tile([1, C], FP32R, name="bp_sb", tag="bp")
        id_sb = wpool.tile([P, P], FP32, name="id_sb", tag="id")
        ones_col = wpool.tile([P, 1], FP32R, name="ones_col", tag="ones_col")
        cst_row = wpool.tile([1, 130], FP32R, name="cst_row", tag="cst_row")

        g_sb = [wpool.tile([P, C], FP32R, name=f"g_sb{m}", tag=f"g{m}") for m in range(NJ)]
        mrow_f = wpool.tile([1, C], FP32, name="mrow_f", tag="mrow_f")
        mrow_r = wpool.tile([1, C], FP32R, name="mrow_r", tag="mrow_r")
        mcol_sb = wpool.tile([P, NJ], FP32R, name="mcol_sb", tag="mcol")
        t_sb = [wpool.tile([P, C], FP32, name=f"t_sb{m}", tag=f"t{m}") for m in range(NJ)]
        t_row_sb = wpool.tile([1, C], FP32, name="t_row_sb", tag="t_row")
        mvt_sb = [wpool.tile([P, C], FP32R, name=f"mvt_sb{p}", tag=f"mvt{p}") for p in range(4)]
        nv_sb = wpool.tile([P, NJ], FP32R, name="nv_sb", tag="nv")
        weff_sb = [wpool.tile([P, C], FP32R, name=f"weff_sb{m}", tag=f"weff{m}") for m in range(NJ)]
        beff_sb = wpool.tile([1, C], FP32R, name="beff_sb", tag="beff")

        for j in range(NJ):
            nc.sync.dma_start(xt_sb[j][:], xt_d[j * P:(j + 1) * P, :])
        for j in range(NJ):
            nc.sync.dma_start(wqa_sb[j][:], wqa_d[j * P:(j + 1) * P, :])
            nc.sync.dma_start(wka_sb[j][:], wka_d[j * P:(j + 1) * P, :])
            nc.sync.dma_start(wp_sb[j][:], wp_d[j * P:(j + 1) * P, :])
        nc.sync.dma_start(wqa_row[:], wqa_d[C:C + 1, :])
        nc.sync.dma_start(wka_row[:], wka_d[C:C + 1, :])
        for p in range(4):
            nc.sync.dma_start(wvt_sb[p][:], wvt_d[p])
            nc.sync.dma_start(bv_sb[p][:], bv_d[p])
        nc.sync.dma_start(bp_sb[:], bp_d[:])
        nc.sync.dma_start(id_sb[:], id_d[:])
        nc.sync.dma_start(ones_col[:], onesc_d[:])
        nc.sync.dma_start(cst_row[:], cst_d[:])
        ones_row = cst_row[0:1, 0:P]
        l_const = cst_row[0:1, P:P + 1]
        one11 = cst_row[0:1, P + 1:P + 2]

        # ---- Phase 1: G_a = [X|1]^T [X|1] accumulated over 32 l-tiles ----
        with tc.tile_pool(name="gps", bufs=1, space="PSUM") as gpool, \
             tc.tile_pool(name="xin", bufs=4) as xpool:
            g_ps = [gpool.tile([P, C], FP32, name=f"g_ps{m}", tag=f"gp{m}") for m in range(NJ)]
            mrow_ps = gpool.tile([1, C], FP32, name="mrow_ps", tag="mrowp")
            for i in range(NT):
                x_t = xpool.tile([P, C], FP32R, name="x_t", tag="x")
                nc.sync.dma_start(x_t[:], x_d[i * P:(i + 1) * P, :])
                for m in range(NJ):
                    nc.tensor.matmul(g_ps[m][:], x_t[:, m * P:(m + 1) * P], x_t[:],
                                     start=(i == 0), stop=(i == NT - 1))
                nc.tensor.matmul(mrow_ps[:], ones_col[:], x_t[:],
                                 start=(i == 0), stop=(i == NT - 1))
            for m in range(NJ):
                if m % 2 == 0:
                    nc.vector.tensor_copy(g_sb[m][:], g_ps[m][:])
                else:
                    nc.scalar.copy(g_sb[m][:], g_ps[m][:])
            nc.vector.tensor_copy(mrow_f[:], mrow_ps[:])
            nc.vector.tensor_copy(mrow_r[:], mrow_ps[:])

        # ---- m as a column: fp32 PE transposes of mrow chunks ----
        with tc.tile_pool(name="mcps", bufs=1, space="PSUM") as mcpool:
            mcol_ps = mcpool.tile([P, NJ], FP32, name="mcol_ps", tag="mcolp")
            for m in range(NJ):
                nc.tensor.transpose(mcol_ps[:, m:m + 1], mrow_f[0:1, m * P:(m + 1) * P],
                                    id_sb[0:1, 0:1])
            nc.vector.tensor_copy(mcol_sb[:], mcol_ps[:])

        # ---- Phase 2a: T = G_a @ Wka   [513, 512] ----
        with tc.tile_pool(name="tps", bufs=1, space="PSUM") as tpool:
            t_ps = [tpool.tile([P, C], FP32, name=f"t_ps{m}", tag=f"tp{m}") for m in range(NJ)]
            t_row_ps = tpool.tile([1, C], FP32, name="t_row_ps", tag="trp")
            for m in range(NJ):
                for j in range(NJ):
                    nc.tensor.matmul(t_ps[m][:], g_sb[j][:, m * P:(m + 1) * P], wka_sb[j][:],
                                     start=(j == 0), stop=False)
                nc.tensor.matmul(t_ps[m][:], mrow_r[0:1, m * P:(m + 1) * P], wka_row[:],
                                 start=False, stop=True)
            for j in range(NJ):
                nc.tensor.matmul(t_row_ps[:], mcol_sb[:, j:j + 1], wka_sb[j][:],
                                 start=(j == 0), stop=False)
            nc.tensor.matmul(t_row_ps[:], l_const, wka_row[:], start=False, stop=True)
            for m in range(NJ):
                if m % 2 == 0:
                    nc.vector.tensor_copy(t_sb[m][:], t_ps[m][:])
                else:
                    nc.scalar.copy(t_sb[m][:], t_ps[m][:])
            nc.vector.tensor_copy(t_row_sb[:], t_row_ps[:])

        # ---- Phase 2b: per group-pair: S -> softmax -> P^T -> MvT, nv ----
        with tc.tile_pool(name="sps", bufs=2, space="PSUM") as spool, \
             tc.tile_pool(name="ptps", bufs=2, space="PSUM") as ptpool, \
             tc.tile_pool(name="mvps", bufs=2, space="PSUM") as mvpool, \
             tc.tile_pool(name="nvps", bufs=1, space="PSUM") as nvpool, \
             tc.tile_pool(name="smx", bufs=2) as smpool:
            nv_ps = nvpool.tile([P, NJ], FP32, name="nv_ps", tag="nvp")
            for p in range(4):
                s_ps = spool.tile([P, HD], FP32, name="s_ps", tag="s")
                for gi in (2 * p, 2 * p + 1):
                    off = (gi % 2) * HD
                    cs = slice(gi * HD, (gi + 1) * HD)
                    for j in range(NJ):
                        nc.tensor.matmul(s_ps[off:off + HD, :], wqa_sb[j][:, cs], t_sb[j][:, cs],
                                         start=(j == 0), stop=False)
                    nc.tensor.matmul(s_ps[off:off + HD, :], wqa_row[0:1, cs], t_row_sb[0:1, cs],
                                     start=False, stop=True)
                neg_mx = smpool.tile([P, 1], FP32, name="neg_mx", tag="mx")
                nc.vector.tensor_reduce(neg_mx[:], s_ps[:], axis=mybir.AxisListType.X,
                                        op=mybir.AluOpType.max, negate=True)
                p_sb = smpool.tile([P, HD], FP32, name="p_sb", tag="p")
                den = smpool.tile([P, 1], FP32, name="den", tag="den")
                nc.scalar.activation(p_sb[:], s_ps[:], mybir.ActivationFunctionType.Exp,
                                     bias=neg_mx[:], scale=1.0, accum_out=den[:])
                rden = smpool.tile([P, 1], FP32, name="rden", tag="rden")
                nc.vector.reciprocal(rden[:], den[:])

                pt_ps = ptpool.tile([HD, P], FP32, name="pt_ps", tag="pt")
                nc.tensor.transpose(pt_ps[:], p_sb[:], id_sb[:])
                pt_sb = smpool.tile([HD, P], FP32R, name="pt_sb", tag="ptsb")
                nc.vector.tensor_copy(pt_sb[:], pt_ps[:])
                pt_sbf = smpool.tile([HD, P], FP32, name="pt_sbf", tag="ptsbf")
                nc.vector.tensor_copy(pt_sbf[:], pt_ps[:])

                # f32r matmuls require tile_position (0, 0): compute each
                # group's MvT with full M=128 (lhsT = whole PT pair) into its
                # own PSUM tile; only rows [off:off+64] are the wanted group.
                mvt_ps_e = mvpool.tile([P, C], FP32, name="mvt_ps_e", tag="mve", bufs=1)
                mvt_ps_o = mvpool.tile([P, C], FP32, name="mvt_ps_o", tag="mvo", bufs=1)
                for gi, mv in ((2 * p, mvt_ps_e), (2 * p + 1, mvt_ps_o)):
                    off = (gi % 2) * HD
                    nc.tensor.matmul(mv[:], pt_sb[:],
                                     wvt_sb[p][:, (gi % 2) * C:(gi % 2 + 1) * C],
                                     start=True, stop=True)
                    nc.tensor.matmul(nv_ps[off:off + HD, p:p + 1], pt_sbf[:, off:off + HD],
                                     bv_sb[p][:, gi % 2:gi % 2 + 1], start=True, stop=True)
                # normalize softmax rows by 1/den during the PSUM->SBUF copy
                nc.scalar.mul(mvt_sb[p][0:HD, :], mvt_ps_e[0:HD, :], rden[0:HD])
                nc.scalar.mul(mvt_sb[p][HD:P, :], mvt_ps_o[HD:P, :], rden[HD:P])
                nc.scalar.mul(nv_sb[:, p:p + 1], nv_ps[:, p:p + 1], rden[:])

        # ---- Phase 2c: W_eff = Mv @ Wp ; b_eff = Wp^T nv + bp ----
        with tc.tile_pool(name="weps", bufs=1, space="PSUM") as wepool:
            weff_ps = [wepool.tile([P, C], FP32, name=f"weff_ps{m}", tag=f"wep{m}") for m in range(NJ)]
            beff_ps = wepool.tile([1, C], FP32, name="beff_ps", tag="bep")
            for m in range(NJ):
                for j in range(NJ):
                    nc.tensor.matmul(weff_ps[m][:], mvt_sb[j][:, m * P:(m + 1) * P], wp_sb[j][:],
                                     start=(j == 0), stop=(j == NJ - 1))
            for j in range(NJ):
                nc.tensor.matmul(beff_ps[:], nv_sb[:, j:j + 1], wp_sb[j][:],
                                 start=(j == 0), stop=False)
            nc.tensor.matmul(beff_ps[:], one11, bp_sb[:], start=False, stop=True)
            for m in range(NJ):
                if m % 2 == 0:
                    nc.vector.tensor_copy(weff_sb[m][:], weff_ps[m][:])
                else:
                    nc.scalar.copy(weff_sb[m][:], weff_ps[m][:])
            nc.vector.tensor_copy(beff_sb[:], beff_ps[:])

        # ---- Phase 3: Y = X @ W_eff + 1 b_eff ----
        with tc.tile_pool(name="yps", bufs=4, space="PSUM") as ypool, \
             tc.tile_pool(name="yout", bufs=4) as yopool:
            for i in range(NT):
                y_ps = ypool.tile([P, C], FP32, name="y_ps", tag="y")
                for j in range(NJ):
                    nc.tensor.matmul(y_ps[:], xt_sb[j][:, i * P:(i + 1) * P], weff_sb[j][:],
                                     start=(j == 0), stop=False)
                nc.tensor.matmul(y_ps[:], ones_row, beff_sb[:], start=False, stop=True)
                y_sb = yopool.tile([P, C], FP32, name="y_sb", tag="ysb")
                if i % 2 == 0:
                    nc.vector.tensor_copy(y_sb[:], y_ps[:])
                else:
                    nc.scalar.copy(y_sb[:], y_ps[:])
                nc.sync.dma_start(y_d[i * P:(i + 1) * P, :], y_sb[:])

    nc.compile()
    return nc


_NC_CACHE = None


def _get_nc():
    global _NC_CACHE
    if _NC_CACHE is None:
        _NC_CACHE = _build_nc()
    return _NC_CACHE


def _prepare_in_maps(x, W_qkv, b_qkv, W_proj, b_proj):
    x = np.ascontiguousarray(np.asarray(x, dtype=np.float32))
    W_qkv = np.asarray(W_qkv, dtype=np.float32)
    b_qkv = np.asarray(b_qkv, dtype=np.float32)
    W_proj = np.ascontiguousarray(np.asarray(W_proj, dtype=np.float32))
    b_proj = np.asarray(b_proj, dtype=np.float32)

    s = float(L) ** -0.5
    wqa = np.concatenate([W_qkv[:, 0:C] * s, b_qkv[None, 0:C] * s], axis=0)
    wka = np.concatenate([W_qkv[:, C:2 * C], b_qkv[None, C:2 * C]], axis=0)
    wv = W_qkv[:, 2 * C:3 * C]
    bv = b_qkv[2 * C:3 * C]
    wvt = np.empty((4, HD, 2 * C), dtype=np.float32)
    bvp = np.empty((4, HD, 2), dtype=np.float32)
    for p in range(4):
        wvt[p, :, 0:C] = wv[:, (2 * p) * HD:(2 * p + 1) * HD].T
        wvt[p, :, C:2 * C] = wv[:, (2 * p + 1) * HD:(2 * p + 2) * HD].T
        bvp[p, :, 0] = bv[(2 * p) * HD:(2 * p + 1) * HD]
        bvp[p, :, 1] = bv[(2 * p + 1) * HD:(2 * p + 2) * HD]
    bp = np.ascontiguousarray(b_proj[None, :])
    id128 = np.eye(P, dtype=np.float32)
    onesc = np.ones((P, 1), dtype=np.float32)
    cst = np.zeros((1, 130), dtype=np.float32)
    cst[0, 0:P] = 1.0
    cst[0, P] = float(L)
    cst[0, P + 1] = 1.0
    xt = np.ascontiguousarray(x.transpose(0, 2, 1))  # [B, C, L]

    shared = {"wqa": np.ascontiguousarray(wqa), "wka": np.ascontiguousarray(wka),
              "wvt": wvt, "bv": bvp, "wp": W_proj, "bp": bp, "id128": id128,
              "onesc": onesc, "cst": cst}
    in_maps = []
    for b in range(B):
        m = dict(shared)
        m["x"] = x[b]
        m["xt"] = xt[b]
        in_maps.append(m)
    return in_maps


def _execute(x, W_qkv, b_qkv, W_proj, b_proj, trace=False, **run_kwargs):
    nc = _get_nc()
    in_maps = _prepare_in_maps(x, W_qkv, b_qkv, W_proj, b_proj)
    res = run_bass_kernel_spmd(nc, in_maps, core_ids=list(range(NCORES)),
                               trace=trace, **run_kwargs)
    out = np.stack([res.results[b]["y"] for b in range(B)], axis=0)
    return out, res


def kernel(x, W_qkv, b_qkv, W_proj, b_proj, size):
    out, _ = _execute(x, W_qkv, b_qkv, W_proj, b_proj)
    return out, size


# revision 7
# speedup vs baseline: 1.0515x; 1.0515x over previous
"""ChannelAttention TRN2 kernel (8 NeuronCores, data-parallel over batch).

Math: for each batch b with X = x[b] [L, C]:
    S_g = Wqa_g^T G_a Wka_g   with G_a = [X|1]^T [X|1] (Gram, contracts over L,
                              absorbs q/k biases; q-scale folded into Wqa)
    P_g = softmax(S_g)        tiny [64, 64] per group
    O   = X Mv + 1 nv^T       with MvT_g = P_g Wv_g^T  (so O never materializes)
    Y   = X W_eff + 1 b_eff   with W_eff = Mv Wp, b_eff = Wp^T nv + bp
so the [L, 3C] qkv intermediate is never computed; per-core work is two
L-sized GEMMs (G = X^T X and Y = X W_eff) plus [512]-scale matrices.

float32r is used for every N=512 matmul (full-rate fp32 streaming; measured
identical numerics to plain fp32 matmul on TRN2). fp32r operands must be
produced as fp32r, hence the dtype choreography on tiles below.
"""

import numpy as np
from contextlib import ExitStack

import concourse.bacc as bacc
import concourse.bass as bass
import concourse.mybir as mybir
import concourse.tile as tile
from concourse.bass_utils import run_bass_kernel_spmd

B, L, C = 8, 4096, 512
GROUPS, HD = 8, 64
P = 128
NT = L // P          # 32 l-tiles
NJ = C // P          # 4 c-chunks
NCORES = 8
FP32 = mybir.dt.float32
FP32R = mybir.dt.float32r


def _build_nc():
    nc = bacc.Bacc("TRN2", target_bir_lowering=False, debug=False, num_devices=NCORES)

    x_d = nc.dram_tensor("x", [L, C], FP32R, kind="ExternalInput").ap()
    xt_d = nc.dram_tensor("xt", [C, L], FP32R, kind="ExternalInput").ap()
    wqa_d = nc.dram_tensor("wqa", [C + 1, C], FP32, kind="ExternalInput").ap()
    wka_d = nc.dram_tensor("wka", [C + 1, C], FP32R, kind="ExternalInput").ap()
    wvt_d = nc.dram_tensor("wvt", [4, HD, 2 * C], FP32R, kind="ExternalInput").ap()
    bv_d = nc.dram_tensor("bv", [4, HD, 2], FP32, kind="ExternalInput").ap()
    wp_d = nc.dram_tensor("wp", [C, C], FP32R, kind="ExternalInput").ap()
    bp_d = nc.dram_tensor("bp", [1, C], FP32R, kind="ExternalInput").ap()
    id_d = nc.dram_tensor("id128", [P, P], FP32, kind="ExternalInput").ap()
    onesc_d = nc.dram_tensor("onesc", [P, 1], FP32R, kind="ExternalInput").ap()
    # cst row: [0:128]=1.0 (Y-bias lhsT), [128]=L, [129]=1.0
    cst_d = nc.dram_tensor("cst", [1, 130], FP32R, kind="ExternalInput").ap()
    y_d = nc.dram_tensor("y", [L, C], FP32, kind="ExternalOutput").ap()

    with tile.TileContext(nc) as tc, ExitStack() as ctx:
        wpool = ctx.enter_context(tc.tile_pool(name="weights", bufs=1))

        xt_sb = [wpool.tile([P, L], FP32R, name=f"xt_sb{j}", tag=f"xt{j}") for j in range(NJ)]
        wqa_sb = [wpool.tile([P, C], FP32, name=f"wqa_sb{j}", tag=f"wqa{j}") for j in range(NJ)]
        wqa_row = wpool.tile([1, C], FP32, name="wqa_row", tag="wqa_row")
        wka_sb = [wpool.tile([P, C], FP32R, name=f"wka_sb{j}", tag=f"wka{j}") for j in range(NJ)]
        wka_row = wpool.tile([1, C], FP32R, name="wka_row", tag="wka_row")
        wvt_sb = [wpool.tile([HD, 2 * C], FP32R, name=f"wvt_sb{p}", tag=f"wvt{p}") for p in range(4)]
        bv_sb = [wpool.tile([HD, 2], FP32, name=f"bv_sb{p}", tag=f"bv{p}") for p in range(4)]
        wp_sb = [wpool.tile([P, C], FP32R, name=f"wp_sb{j}", tag=f"wp{j}") for j in range(NJ)]
        bp_sb = wpool.tile([1, C], FP32R, name="bp_sb", tag="bp")
        id_sb = wpool.tile([P, P], FP32, name="id_sb", tag="id")
        ones_col = wpool.tile([P, 1], FP32R, name="ones_col", tag="ones_col")
        cst_row = wpool.tile([1, 130], FP32R, name="cst_row", tag="cst_row")

        g_sb = [wpool.tile([P, C], FP32R, name=f"g_sb{m}", tag=f"g{m}") for m in range(NJ)]
        mrow_f = wpool.tile([1, C], FP32, name="mrow_f", tag="mrow_f")
        mrow_r = wpool.tile([1, C], FP32R, name="mrow_r", tag="mrow_r")
        mcol_sb = wpool.tile([P, NJ], FP32R, name="mcol_sb", tag="mcol")
        t_sb = [wpool.tile([P, C], FP32, name=f"t_sb{m}", tag=f"t{m}") for m in range(NJ)]
        t_row_sb = wpool.tile([1, C], FP32, name="t_row_sb", tag="t_row")
        mvt_sb = [wpool.tile([P, C], FP32R, name=f"mvt_sb{p}", tag=f"mvt{p}") for p in range(4)]
        nv_sb = wpool.tile([P, NJ], FP32R, name="nv_sb", tag="nv")
        weff_sb = [wpool.tile([P, C], FP32R, name=f"weff_sb{m}", tag=f"weff{m}") for m in range(NJ)]
        beff_sb = wpool.tile([1, C], FP32R, name="beff_sb", tag="beff")

        # Only the tiny constants load ahead of the x stream (ones_col gates
        # the first G-phase matmul). Bulk weights and xt are emitted later so
        # the single HWDGE queue serves phase-1 x tiles first.
        nc.sync.dma_start(ones_col[:], onesc_d[:])
        nc.sync.dma_start(cst_row[:], cst_d[:])
        nc.sync.dma_start(id_sb[:], id_d[:])
        ones_row = cst_row[0:1, 0:P]
        l_const = cst_row[0:1, P:P + 1]
        one11 = cst_row[0:1, P + 1:P + 2]

        # ---- Phase 1: G_a = [X|1]^T [X|1] accumulated over 32 l-tiles ----
        with tc.tile_pool(name="gps", bufs=1, space="PSUM") as gpool, \
             tc.tile_pool(name="xin", bufs=4) as xpool:
            g_ps = [gpool.tile([P, C], FP32, name=f"g_ps{m}", tag=f"gp{m}") for m in range(NJ)]
            mrow_ps = gpool.tile([1, C], FP32, name="mrow_ps", tag="mrowp")
            for i in range(NT):
                x_t = xpool.tile([P, C], FP32R, name="x_t", tag="x")
                nc.sync.dma_start(x_t[:], x_d[i * P:(i + 1) * P, :])
                for m in range(NJ):
                    nc.tensor.matmul(g_ps[m][:], x_t[:, m * P:(m + 1) * P], x_t[:],
                                     start=(i == 0), stop=(i == NT - 1))
                nc.tensor.matmul(mrow_ps[:], ones_col[:], x_t[:],
                                 start=(i == 0), stop=(i == NT - 1))
                if i == 0:
                    # weight loads: emitted after the first x tile so they
                    # queue behind it; they arrive during G accumulation.
                    for j in range(NJ):
                        nc.sync.dma_start(wka_sb[j][:], wka_d[j * P:(j + 1) * P, :])
                        nc.sync.dma_start(wqa_sb[j][:], wqa_d[j * P:(j + 1) * P, :])
                        nc.sync.dma_start(wp_sb[j][:], wp_d[j * P:(j + 1) * P, :])
                    nc.sync.dma_start(wqa_row[:], wqa_d[C:C + 1, :])
                    nc.sync.dma_start(wka_row[:], wka_d[C:C + 1, :])
                    for p in range(4):
                        nc.sync.dma_start(wvt_sb[p][:], wvt_d[p])
                        nc.sync.dma_start(bv_sb[p][:], bv_d[p])
                    nc.sync.dma_start(bp_sb[:], bp_d[:])
                    # xt bulk prefetch on the SWDGE (gpsimd) queues — runs in
                    # parallel with the sync-queue x stream; needed in phase 3.
                    for j in range(NJ):
                        nc.gpsimd.dma_start(xt_sb[j][:], xt_d[j * P:(j + 1) * P, :])
            for m in range(NJ):
                if m % 2 == 0:
                    nc.vector.tensor_copy(g_sb[m][:], g_ps[m][:])
                else:
                    nc.scalar.copy(g_sb[m][:], g_ps[m][:])
            nc.vector.tensor_copy(mrow_f[:], mrow_ps[:])
            nc.vector.tensor_copy(mrow_r[:], mrow_ps[:])

        # ---- m as a column: fp32 PE transposes of mrow chunks ----
        with tc.tile_pool(name="mcps", bufs=1, space="PSUM") as mcpool:
            mcol_ps = mcpool.tile([P, NJ], FP32, name="mcol_ps", tag="mcolp")
            for m in range(NJ):
                nc.tensor.transpose(mcol_ps[:, m:m + 1], mrow_f[0:1, m * P:(m + 1) * P],
                                    id_sb[0:1, 0:1])
            nc.vector.tensor_copy(mcol_sb[:], mcol_ps[:])

        # ---- Phase 2a: T = G_a @ Wka   [513, 512] ----
        with tc.tile_pool(name="tps", bufs=1, space="PSUM") as tpool:
            t_ps = [tpool.tile([P, C], FP32, name=f"t_ps{m}", tag=f"tp{m}") for m in range(NJ)]
            t_row_ps = tpool.tile([1, C], FP32, name="t_row_ps", tag="trp")
            for m in range(NJ):
                for j in range(NJ):
                    nc.tensor.matmul(t_ps[m][:], g_sb[j][:, m * P:(m + 1) * P], wka_sb[j][:],
                                     start=(j == 0), stop=False)
                nc.tensor.matmul(t_ps[m][:], mrow_r[0:1, m * P:(m + 1) * P], wka_row[:],
                                 start=False, stop=True)
            for j in range(NJ):
                nc.tensor.matmul(t_row_ps[:], mcol_sb[:, j:j + 1], wka_sb[j][:],
                                 start=(j == 0), stop=False)
            nc.tensor.matmul(t_row_ps[:], l_const, wka_row[:], start=False, stop=True)
            for m in range(NJ):
                if m % 2 == 0:
                    nc.vector.tensor_copy(t_sb[m][:], t_ps[m][:])
                else:
                    nc.scalar.copy(t_sb[m][:], t_ps[m][:])
            nc.vector.tensor_copy(t_row_sb[:], t_row_ps[:])

        # ---- Phase 2b: per group-pair: S -> softmax -> P^T -> MvT, nv ----
        with tc.tile_pool(name="sps", bufs=2, space="PSUM") as spool, \
             tc.tile_pool(name="ptps", bufs=2, space="PSUM") as ptpool, \
             tc.tile_pool(name="mvps", bufs=2, space="PSUM") as mvpool, \
             tc.tile_pool(name="nvps", bufs=1, space="PSUM") as nvpool, \
             tc.tile_pool(name="smx", bufs=2) as smpool:
            nv_ps = nvpool.tile([P, NJ], FP32, name="nv_ps", tag="nvp")
            for p in range(4):
                s_ps = spool.tile([P, HD], FP32, name="s_ps", tag="s")
                for gi in (2 * p, 2 * p + 1):
                    off = (gi % 2) * HD
                    cs = slice(gi * HD, (gi + 1) * HD)
                    for j in range(NJ):
                        nc.tensor.matmul(s_ps[off:off + HD, :], wqa_sb[j][:, cs], t_sb[j][:, cs],
                                         start=(j == 0), stop=False)
                    nc.tensor.matmul(s_ps[off:off + HD, :], wqa_row[0:1, cs], t_row_sb[0:1, cs],
                                     start=False, stop=True)
                neg_mx = smpool.tile([P, 1], FP32, name="neg_mx", tag="mx")
                nc.vector.tensor_reduce(neg_mx[:], s_ps[:], axis=mybir.AxisListType.X,
                                        op=mybir.AluOpType.max, negate=True)
                p_sb = smpool.tile([P, HD], FP32, name="p_sb", tag="p")
                den = smpool.tile([P, 1], FP32, name="den", tag="den")
                nc.scalar.activation(p_sb[:], s_ps[:], mybir.ActivationFunctionType.Exp,
                                     bias=neg_mx[:], scale=1.0, accum_out=den[:])
                rden = smpool.tile([P, 1], FP32, name="rden", tag="rden")
                nc.vector.reciprocal(rden[:], den[:])

                pt_ps = ptpool.tile([HD, P], FP32, name="pt_ps", tag="pt")
                nc.tensor.transpose(pt_ps[:], p_sb[:], id_sb[:])
                pt_sb = smpool.tile([HD, P], FP32R, name="pt_sb", tag="ptsb")
                nc.vector.tensor_copy(pt_sb[:], pt_ps[:])
                pt_sbf = smpool.tile([HD, P], FP32, name="pt_sbf", tag="ptsbf")
                nc.vector.tensor_copy(pt_sbf[:], pt_ps[:])

                # f32r matmuls require tile_position (0, 0): compute each
                # group's MvT with full M=128 (lhsT = whole PT pair) into its
                # own PSUM tile; only rows [off:off+64] are the wanted group.
                mvt_ps_e = mvpool.tile([P, C], FP32, name="mvt_ps_e", tag="mve", bufs=1)
                mvt_ps_o = mvpool.tile([P, C], FP32, name="mvt_ps_o", tag="mvo", bufs=1)
                for gi, mv in ((2 * p, mvt_ps_e), (2 * p + 1, mvt_ps_o)):
                    off = (gi % 2) * HD
                    nc.tensor.matmul(mv[:], pt_sb[:],
                                     wvt_sb[p][:, (gi % 2) * C:(gi % 2 + 1) * C],
                                     start=True, stop=True)
                    nc.tensor.matmul(nv_ps[off:off + HD, p:p + 1], pt_sbf[:, off:off + HD],
                                     bv_sb[p][:, gi % 2:gi % 2 + 1], start=True, stop=True)
                # normalize softmax rows by 1/den during the PSUM->SBUF copy
                nc.scalar.mul(mvt_sb[p][0:HD, :], mvt_ps_e[0:HD, :], rden[0:HD])
                nc.scalar.mul(mvt_sb[p][HD:P, :], mvt_ps_o[HD:P, :], rden[HD:P])
                nc.scalar.mul(nv_sb[:, p:p + 1], nv_ps[:, p:p + 1], rden[:])

        # ---- Phase 2c: W_eff = Mv @ Wp ; b_eff = Wp^T nv + bp ----
        with tc.tile_pool(name="weps", bufs=1, space="PSUM") as wepool:
            weff_ps = [wepool.tile([P, C], FP32, name=f"weff_ps{m}", tag=f"wep{m}") for m in range(NJ)]
            beff_ps = wepool.tile([1, C], FP32, name="beff_ps", tag="bep")
            for m in range(NJ):
                for j in range(NJ):
                    nc.tensor.matmul(weff_ps[m][:], mvt_sb[j][:, m * P:(m + 1) * P], wp_sb[j][:],
                                     start=(j == 0), stop=(j == NJ - 1))
            for j in range(NJ):
                nc.tensor.matmul(beff_ps[:], nv_sb[:, j:j + 1], wp_sb[j][:],
                                 start=(j == 0), stop=False)
            nc.tensor.matmul(beff_ps[:], one11, bp_sb[:], start=False, stop=True)
            for m in range(NJ):
                if m % 2 == 0:
                    nc.vector.tensor_copy(weff_sb[m][:], weff_ps[m][:])
                else:
                    nc.scalar.copy(weff_sb[m][:], weff_ps[m][:])
            nc.vector.tensor_copy(beff_sb[:], beff_ps[:])

        # ---- Phase 3: Y = X @ W_eff + 1 b_eff ----
        with tc.tile_pool(name="yps", bufs=4, space="PSUM") as ypool, \
             tc.tile_pool(name="yout", bufs=4) as yopool:
            for i in range(NT):
                y_ps = ypool.tile([P, C], FP32, name="y_ps", tag="y")
                for j in range(NJ):
                    nc.tensor.matmul(y_ps[:], xt_sb[j][:, i * P:(i + 1) * P], weff_sb[j][:],
                                     start=(j == 0), stop=False)
                nc.tensor.matmul(y_ps[:], ones_row, beff_sb[:], start=False, stop=True)
                y_sb = yopool.tile([P, C], FP32, name="y_sb", tag="ysb")
                if i % 2 == 0:
                    nc.vector.tensor_copy(y_sb[:], y_ps[:])
                else:
                    nc.scalar.copy(y_sb[:], y_ps[:])
                nc.sync.dma_start(y_d[i * P:(i + 1) * P, :], y_sb[:])

    nc.compile()
    return nc


_NC_CACHE = None


def _get_nc():
    global _NC_CACHE
    if _NC_CACHE is None:
        _NC_CACHE = _build_nc()
    return _NC_CACHE


def _prepare_in_maps(x, W_qkv, b_qkv, W_proj, b_proj):
    x = np.ascontiguousarray(np.asarray(x, dtype=np.float32))
    W_qkv = np.asarray(W_qkv, dtype=np.float32)
    b_qkv = np.asarray(b_qkv, dtype=np.float32)
    W_proj = np.ascontiguousarray(np.asarray(W_proj, dtype=np.float32))
    b_proj = np.asarray(b_proj, dtype=np.float32)

    s = float(L) ** -0.5
    wqa = np.concatenate([W_qkv[:, 0:C] * s, b_qkv[None, 0:C] * s], axis=0)
    wka = np.concatenate([W_qkv[:, C:2 * C], b_qkv[None, C:2 * C]], axis=0)
    wv = W_qkv[:, 2 * C:3 * C]
    bv = b_qkv[2 * C:3 * C]
    wvt = np.empty((4, HD, 2 * C), dtype=np.float32)
    bvp = np.empty((4, HD, 2), dtype=np.float32)
    for p in range(4):
        wvt[p, :, 0:C] = wv[:, (2 * p) * HD:(2 * p + 1) * HD].T
        wvt[p, :, C:2 * C] = wv[:, (2 * p + 1) * HD:(2 * p + 2) * HD].T
        bvp[p, :, 0] = bv[(2 * p) * HD:(2 * p + 1) * HD]
        bvp[p, :, 1] = bv[(2 * p + 1) * HD:(2 * p + 2) * HD]
    bp = np.ascontiguousarray(b_proj[None, :])
    id128 = np.eye(P, dtype=np.float32)
    onesc = np.ones((P, 1), dtype=np.float32)
    cst = np.zeros((1, 130), dtype=np.float32)
    cst[0, 0:P] = 1.0
    cst[0, P] = float(L)
    cst[0, P + 1] = 1.0
    xt = np.ascontiguousarray(x.transpose(0, 2, 1))  # [B, C, L]

    shared = {"wqa": np.ascontiguousarray(wqa), "wka": np.ascontiguousarray(wka),
              "wvt": wvt, "bv": bvp, "wp": W_proj, "bp": bp, "id128": id128,
              "onesc": onesc, "cst": cst}
    in_maps = []
    for b in range(B):
        m = dict(shared)
        m["x"] = x[b]
        m["xt"] = xt[b]
        in_maps.append(m)
    return in_maps


def _execute(x, W_qkv, b_qkv, W_proj, b_proj, trace=False, **run_kwargs):
    nc = _get_nc()
    in_maps = _prepare_in_maps(x, W_qkv, b_qkv, W_proj, b_proj)
    res = run_bass_kernel_spmd(nc, in_maps, core_ids=list(range(NCORES)),
                               trace=trace, **run_kwargs)
    out = np.stack([res.results[b]["y"] for b in range(B)], axis=0)
    return out, res


def kernel(x, W_qkv, b_qkv, W_proj, b_proj, size):
    out, _ = _execute(x, W_qkv, b_qkv, W_proj, b_proj)
    return out, size


# revision 8
# speedup vs baseline: 1.0645x; 1.0124x over previous
"""ChannelAttention TRN2 kernel (8 NeuronCores, data-parallel over batch).

Math: for each batch b with X = x[b] [L, C]:
    S_g = Wqa_g^T G_a Wka_g   with G_a = [X|1]^T [X|1] (Gram, contracts over L,
                              absorbs q/k biases; q-scale folded into Wqa)
    P_g = softmax(S_g)        tiny [64, 64] per group
    O   = X Mv + 1 nv^T       with MvT_g = P_g Wv_g^T  (so O never materializes)
    Y   = X W_eff + 1 b_eff   with W_eff = Mv Wp, b_eff = Wp^T nv + bp
so the [L, 3C] qkv intermediate is never computed; per-core work is two
L-sized GEMMs (G = X^T X and Y = X W_eff) plus [512]-scale matrices.

float32r is used for every N=512 matmul (full-rate fp32 streaming; measured
identical numerics to plain fp32 matmul on TRN2). fp32r operands must be
produced as fp32r, hence the dtype choreography on tiles below.
"""

import numpy as np
from contextlib import ExitStack

import concourse.bacc as bacc
import concourse.bass as bass
import concourse.mybir as mybir
import concourse.tile as tile
from concourse.bass_utils import run_bass_kernel_spmd

B, L, C = 8, 4096, 512
GROUPS, HD = 8, 64
P = 128
NT = L // P          # 32 l-tiles
NJ = C // P          # 4 c-chunks
NCORES = 8
FP32 = mybir.dt.float32
FP32R = mybir.dt.float32r


def _build_nc():
    nc = bacc.Bacc("TRN2", target_bir_lowering=False, debug=False, num_devices=NCORES)

    x_d = nc.dram_tensor("x", [L, C], FP32R, kind="ExternalInput").ap()
    xt_d = nc.dram_tensor("xt", [C, L], FP32R, kind="ExternalInput").ap()
    wqa_d = nc.dram_tensor("wqa", [C + 1, C], FP32, kind="ExternalInput").ap()
    wka_d = nc.dram_tensor("wka", [C + 1, C], FP32R, kind="ExternalInput").ap()
    wvt_d = nc.dram_tensor("wvt", [4, HD, 2 * C], FP32R, kind="ExternalInput").ap()
    bv_d = nc.dram_tensor("bv", [4, HD, 2], FP32, kind="ExternalInput").ap()
    wp_d = nc.dram_tensor("wp", [C, C], FP32R, kind="ExternalInput").ap()
    bp_d = nc.dram_tensor("bp", [1, C], FP32R, kind="ExternalInput").ap()
    id_d = nc.dram_tensor("id128", [P, P], FP32, kind="ExternalInput").ap()
    onesc_d = nc.dram_tensor("onesc", [P, 1], FP32R, kind="ExternalInput").ap()
    # cst row: [0:128]=1.0 (Y-bias lhsT), [128]=L, [129]=1.0
    cst_d = nc.dram_tensor("cst", [1, 130], FP32R, kind="ExternalInput").ap()
    y_d = nc.dram_tensor("y", [L, C], FP32, kind="ExternalOutput").ap()

    with tile.TileContext(nc) as tc, ExitStack() as ctx:
        wpool = ctx.enter_context(tc.tile_pool(name="weights", bufs=1))

        xt_sb = [wpool.tile([P, L], FP32R, name=f"xt_sb{j}", tag=f"xt{j}") for j in range(NJ)]
        wqa_sb = [wpool.tile([P, C], FP32, name=f"wqa_sb{j}", tag=f"wqa{j}") for j in range(NJ)]
        wqa_row = wpool.tile([1, C], FP32, name="wqa_row", tag="wqa_row")
        wka_sb = [wpool.tile([P, C], FP32R, name=f"wka_sb{j}", tag=f"wka{j}") for j in range(NJ)]
        wka_row = wpool.tile([1, C], FP32R, name="wka_row", tag="wka_row")
        wvt_sb = [wpool.tile([HD, 2 * C], FP32R, name=f"wvt_sb{p}", tag=f"wvt{p}") for p in range(4)]
        bv_sb = [wpool.tile([HD, 2], FP32, name=f"bv_sb{p}", tag=f"bv{p}") for p in range(4)]
        wp_sb = [wpool.tile([P, C], FP32R, name=f"wp_sb{j}", tag=f"wp{j}") for j in range(NJ)]
        bp_sb = wpool.tile([1, C], FP32R, name="bp_sb", tag="bp")
        id_sb = wpool.tile([P, P], FP32, name="id_sb", tag="id")
        ones_col = wpool.tile([P, 1], FP32R, name="ones_col", tag="ones_col")
        cst_row = wpool.tile([1, 130], FP32R, name="cst_row", tag="cst_row")

        g_sb = [wpool.tile([P, C], FP32R, name=f"g_sb{m}", tag=f"g{m}") for m in range(NJ)]
        mrow_f = wpool.tile([1, C], FP32, name="mrow_f", tag="mrow_f")
        mrow_r = wpool.tile([1, C], FP32R, name="mrow_r", tag="mrow_r")
        mcol_sb = wpool.tile([P, NJ], FP32R, name="mcol_sb", tag="mcol")
        t_sb = [wpool.tile([P, C], FP32, name=f"t_sb{m}", tag=f"t{m}") for m in range(NJ)]
        t_row_sb = wpool.tile([1, C], FP32, name="t_row_sb", tag="t_row")
        mvt_sb = [wpool.tile([P, C], FP32R, name=f"mvt_sb{p}", tag=f"mvt{p}") for p in range(4)]
        nv_sb = wpool.tile([P, NJ], FP32R, name="nv_sb", tag="nv")
        weff_sb = [wpool.tile([P, C], FP32R, name=f"weff_sb{m}", tag=f"weff{m}") for m in range(NJ)]
        beff_sb = wpool.tile([1, C], FP32R, name="beff_sb", tag="beff")

        # Only the tiny constants load ahead of the x stream (ones_col gates
        # the first G-phase matmul). Bulk weights and xt are emitted later so
        # the single HWDGE queue serves phase-1 x tiles first.
        nc.sync.dma_start(ones_col[:], onesc_d[:])
        nc.sync.dma_start(cst_row[:], cst_d[:])
        nc.sync.dma_start(id_sb[:], id_d[:])
        ones_row = cst_row[0:1, 0:P]
        l_const = cst_row[0:1, P:P + 1]
        one11 = cst_row[0:1, P + 1:P + 2]

        # ---- Phase 1: G_a = [X|1]^T [X|1] accumulated over 32 l-tiles ----
        with tc.tile_pool(name="gps", bufs=1, space="PSUM") as gpool, \
             tc.tile_pool(name="xin", bufs=4) as xpool:
            g_ps = [gpool.tile([P, C], FP32, name=f"g_ps{m}", tag=f"gp{m}") for m in range(NJ)]
            mrow_ps = gpool.tile([1, C], FP32, name="mrow_ps", tag="mrowp")
            for i in range(NT):
                x_t = xpool.tile([P, C], FP32R, name="x_t", tag="x")
                nc.sync.dma_start(x_t[:], x_d[i * P:(i + 1) * P, :])
                for m in range(NJ):
                    nc.tensor.matmul(g_ps[m][:], x_t[:, m * P:(m + 1) * P], x_t[:],
                                     start=(i == 0), stop=(i == NT - 1))
                nc.tensor.matmul(mrow_ps[:], ones_col[:], x_t[:],
                                 start=(i == 0), stop=(i == NT - 1))
                if i == 0:
                    # xt bulk prefetch on the SWDGE (gpsimd) queues — runs in
                    # parallel with the sync-queue x stream; needed in phase 3.
                    for j in range(NJ):
                        nc.gpsimd.dma_start(xt_sb[j][:], xt_d[j * P:(j + 1) * P, :])
            # weight loads: emitted after the whole x stream so they queue
            # behind it on the sync HWDGE; they arrive during G accumulation
            # and are first needed in phase 2.
            for j in range(NJ):
                nc.sync.dma_start(wka_sb[j][:], wka_d[j * P:(j + 1) * P, :])
                nc.sync.dma_start(wqa_sb[j][:], wqa_d[j * P:(j + 1) * P, :])
                nc.sync.dma_start(wp_sb[j][:], wp_d[j * P:(j + 1) * P, :])
            nc.sync.dma_start(wqa_row[:], wqa_d[C:C + 1, :])
            nc.sync.dma_start(wka_row[:], wka_d[C:C + 1, :])
            for p in range(4):
                nc.sync.dma_start(wvt_sb[p][:], wvt_d[p])
                nc.sync.dma_start(bv_sb[p][:], bv_d[p])
            nc.sync.dma_start(bp_sb[:], bp_d[:])
            for m in range(NJ):
                if m % 2 == 0:
                    nc.vector.tensor_copy(g_sb[m][:], g_ps[m][:])
                else:
                    nc.scalar.copy(g_sb[m][:], g_ps[m][:])
            nc.vector.tensor_copy(mrow_f[:], mrow_ps[:])
            nc.vector.tensor_copy(mrow_r[:], mrow_ps[:])

        # ---- m as a column: fp32 PE transposes of mrow chunks ----
        with tc.tile_pool(name="mcps", bufs=1, space="PSUM") as mcpool:
            mcol_ps = mcpool.tile([P, NJ], FP32, name="mcol_ps", tag="mcolp")
            for m in range(NJ):
                nc.tensor.transpose(mcol_ps[:, m:m + 1], mrow_f[0:1, m * P:(m + 1) * P],
                                    id_sb[0:1, 0:1])
            nc.vector.tensor_copy(mcol_sb[:], mcol_ps[:])

        # ---- Phase 2a: T = G_a @ Wka   [513, 512] ----
        with tc.tile_pool(name="tps", bufs=1, space="PSUM") as tpool:
            t_ps = [tpool.tile([P, C], FP32, name=f"t_ps{m}", tag=f"tp{m}") for m in range(NJ)]
            t_row_ps = tpool.tile([1, C], FP32, name="t_row_ps", tag="trp")
            for m in range(NJ):
                for j in range(NJ):
                    nc.tensor.matmul(t_ps[m][:], g_sb[j][:, m * P:(m + 1) * P], wka_sb[j][:],
                                     start=(j == 0), stop=False)
                nc.tensor.matmul(t_ps[m][:], mrow_r[0:1, m * P:(m + 1) * P], wka_row[:],
                                 start=False, stop=True)
            for j in range(NJ):
                nc.tensor.matmul(t_row_ps[:], mcol_sb[:, j:j + 1], wka_sb[j][:],
                                 start=(j == 0), stop=False)
            nc.tensor.matmul(t_row_ps[:], l_const, wka_row[:], start=False, stop=True)
            for m in range(NJ):
                if m % 2 == 0:
                    nc.vector.tensor_copy(t_sb[m][:], t_ps[m][:])
                else:
                    nc.scalar.copy(t_sb[m][:], t_ps[m][:])
            nc.vector.tensor_copy(t_row_sb[:], t_row_ps[:])

        # ---- Phase 2b: per group-pair: S -> softmax -> P^T -> MvT, nv ----
        with tc.tile_pool(name="sps", bufs=2, space="PSUM") as spool, \
             tc.tile_pool(name="ptps", bufs=2, space="PSUM") as ptpool, \
             tc.tile_pool(name="mvps", bufs=2, space="PSUM") as mvpool, \
             tc.tile_pool(name="nvps", bufs=1, space="PSUM") as nvpool, \
             tc.tile_pool(name="smx", bufs=2) as smpool:
            nv_ps = nvpool.tile([P, NJ], FP32, name="nv_ps", tag="nvp")
            for p in range(4):
                s_ps = spool.tile([P, HD], FP32, name="s_ps", tag="s")
                for gi in (2 * p, 2 * p + 1):
                    off = (gi % 2) * HD
                    cs = slice(gi * HD, (gi + 1) * HD)
                    for j in range(NJ):
                        nc.tensor.matmul(s_ps[off:off + HD, :], wqa_sb[j][:, cs], t_sb[j][:, cs],
                                         start=(j == 0), stop=False)
                    nc.tensor.matmul(s_ps[off:off + HD, :], wqa_row[0:1, cs], t_row_sb[0:1, cs],
                                     start=False, stop=True)
                neg_mx = smpool.tile([P, 1], FP32, name="neg_mx", tag="mx")
                nc.vector.tensor_reduce(neg_mx[:], s_ps[:], axis=mybir.AxisListType.X,
                                        op=mybir.AluOpType.max, negate=True)
                p_sb = smpool.tile([P, HD], FP32, name="p_sb", tag="p")
                den = smpool.tile([P, 1], FP32, name="den", tag="den")
                nc.scalar.activation(p_sb[:], s_ps[:], mybir.ActivationFunctionType.Exp,
                                     bias=neg_mx[:], scale=1.0, accum_out=den[:])
                rden = smpool.tile([P, 1], FP32, name="rden", tag="rden")
                nc.vector.reciprocal(rden[:], den[:])

                pt_ps = ptpool.tile([HD, P], FP32, name="pt_ps", tag="pt")
                nc.tensor.transpose(pt_ps[:], p_sb[:], id_sb[:])
                pt_sb = smpool.tile([HD, P], FP32R, name="pt_sb", tag="ptsb")
                nc.vector.tensor_copy(pt_sb[:], pt_ps[:])
                pt_sbf = smpool.tile([HD, P], FP32, name="pt_sbf", tag="ptsbf")
                nc.vector.tensor_copy(pt_sbf[:], pt_ps[:])

                # f32r matmuls require tile_position (0, 0): compute each
                # group's MvT with full M=128 (lhsT = whole PT pair) into its
                # own PSUM tile; only rows [off:off+64] are the wanted group.
                mvt_ps_e = mvpool.tile([P, C], FP32, name="mvt_ps_e", tag="mve", bufs=1)
                mvt_ps_o = mvpool.tile([P, C], FP32, name="mvt_ps_o", tag="mvo", bufs=1)
                for gi, mv in ((2 * p, mvt_ps_e), (2 * p + 1, mvt_ps_o)):
                    off = (gi % 2) * HD
                    nc.tensor.matmul(mv[:], pt_sb[:],
                                     wvt_sb[p][:, (gi % 2) * C:(gi % 2 + 1) * C],
                                     start=True, stop=True)
                    nc.tensor.matmul(nv_ps[off:off + HD, p:p + 1], pt_sbf[:, off:off + HD],
                                     bv_sb[p][:, gi % 2:gi % 2 + 1], start=True, stop=True)
                # normalize softmax rows by 1/den during the PSUM->SBUF copy
                nc.scalar.mul(mvt_sb[p][0:HD, :], mvt_ps_e[0:HD, :], rden[0:HD])
                nc.scalar.mul(mvt_sb[p][HD:P, :], mvt_ps_o[HD:P, :], rden[HD:P])
                nc.scalar.mul(nv_sb[:, p:p + 1], nv_ps[:, p:p + 1], rden[:])

        # ---- Phase 2c: W_eff = Mv @ Wp ; b_eff = Wp^T nv + bp ----
        with tc.tile_pool(name="weps", bufs=1, space="PSUM") as wepool:
            weff_ps = [wepool.tile([P, C], FP32, name=f"weff_ps{m}", tag=f"wep{m}") for m in range(NJ)]
            beff_ps = wepool.tile([1, C], FP32, name="beff_ps", tag="bep")
            for m in range(NJ):
                for j in range(NJ):
                    nc.tensor.matmul(weff_ps[m][:], mvt_sb[j][:, m * P:(m + 1) * P], wp_sb[j][:],
                                     start=(j == 0), stop=(j == NJ - 1))
            for j in range(NJ):
                nc.tensor.matmul(beff_ps[:], nv_sb[:, j:j + 1], wp_sb[j][:],
                                 start=(j == 0), stop=False)
            nc.tensor.matmul(beff_ps[:], one11, bp_sb[:], start=False, stop=True)
            for m in range(NJ):
                if m % 2 == 0:
                    nc.vector.tensor_copy(weff_sb[m][:], weff_ps[m][:])
                else:
                    nc.scalar.copy(weff_sb[m][:], weff_ps[m][:])
            nc.vector.tensor_copy(beff_sb[:], beff_ps[:])

        # ---- Phase 3: Y = X @ W_eff + 1 b_eff ----
        with tc.tile_pool(name="yps", bufs=4, space="PSUM") as ypool, \
             tc.tile_pool(name="yout", bufs=4) as yopool:
            for i in range(NT):
                y_ps = ypool.tile([P, C], FP32, name="y_ps", tag="y")
                for j in range(NJ):
                    nc.tensor.matmul(y_ps[:], xt_sb[j][:, i * P:(i + 1) * P], weff_sb[j][:],
                                     start=(j == 0), stop=False)
                nc.tensor.matmul(y_ps[:], ones_row, beff_sb[:], start=False, stop=True)
                y_sb = yopool.tile([P, C], FP32, name="y_sb", tag="ysb")
                if i % 2 == 0:
                    nc.vector.tensor_copy(y_sb[:], y_ps[:])
                else:
                    nc.scalar.copy(y_sb[:], y_ps[:])
                nc.sync.dma_start(y_d[i * P:(i + 1) * P, :], y_sb[:])

    nc.compile()
    return nc


_NC_CACHE = None


def _get_nc():
    global _NC_CACHE
    if _NC_CACHE is None:
        _NC_CACHE = _build_nc()
    return _NC_CACHE


def _prepare_in_maps(x, W_qkv, b_qkv, W_proj, b_proj):
    x = np.ascontiguousarray(np.asarray(x, dtype=np.float32))
    W_qkv = np.asarray(W_qkv, dtype=np.float32)
    b_qkv = np.asarray(b_qkv, dtype=np.float32)
    W_proj = np.ascontiguousarray(np.asarray(W_proj, dtype=np.float32))
    b_proj = np.asarray(b_proj, dtype=np.float32)

    s = float(L) ** -0.5
    wqa = np.concatenate([W_qkv[:, 0:C] * s, b_qkv[None, 0:C] * s], axis=0)
    wka = np.concatenate([W_qkv[:, C:2 * C], b_qkv[None, C:2 * C]], axis=0)
    wv = W_qkv[:, 2 * C:3 * C]
    bv = b_qkv[2 * C:3 * C]
    wvt = np.empty((4, HD, 2 * C), dtype=np.float32)
    bvp = np.empty((4, HD, 2), dtype=np.float32)
    for p in range(4):
        wvt[p, :, 0:C] = wv[:, (2 * p) * HD:(2 * p + 1) * HD].T
        wvt[p, :, C:2 * C] = wv[:, (2 * p + 1) * HD:(2 * p + 2) * HD].T
        bvp[p, :, 0] = bv[(2 * p) * HD:(2 * p + 1) * HD]
        bvp[p, :, 1] = bv[(2 * p + 1) * HD:(2 * p + 2) * HD]
    bp = np.ascontiguousarray(b_proj[None, :])
    id128 = np.eye(P, dtype=np.float32)
    onesc = np.ones((P, 1), dtype=np.float32)
    cst = np.zeros((1, 130), dtype=np.float32)
    cst[0, 0:P] = 1.0
    cst[0, P] = float(L)
    cst[0, P + 1] = 1.0
    xt = np.ascontiguousarray(x.transpose(0, 2, 1))  # [B, C, L]

    shared = {"wqa": np.ascontiguousarray(wqa), "wka": np.ascontiguousarray(wka),
              "wvt": wvt, "bv": bvp, "wp": W_proj, "bp": bp, "id128": id128,
              "onesc": onesc, "cst": cst}
    in_maps = []
    for b in range(B):
        m = dict(shared)
        m["x"] = x[b]
        m["xt"] = xt[b]
        in_maps.append(m)
    return in_maps


def _execute(x, W_qkv, b_qkv, W_proj, b_proj, trace=False, **run_kwargs):
    nc = _get_nc()
    in_maps = _prepare_in_maps(x, W_qkv, b_qkv, W_proj, b_proj)
    res = run_bass_kernel_spmd(nc, in_maps, core_ids=list(range(NCORES)),
                               trace=trace, **run_kwargs)
    out = np.stack([res.results[b]["y"] for b in range(B)], axis=0)
    return out, res


def kernel(x, W_qkv, b_qkv, W_proj, b_proj, size):
    out, _ = _execute(x, W_qkv, b_qkv, W_proj, b_proj)
    return out, size
